# revision 13
# baseline (speedup 1.0000x reference)
"""Trainium2 Bass kernel for nn_AttentionBlock (modconv -> self-attn -> cross-attn
-> top1-MoE -> modconv), SPMD over 8 NeuronCores: core = (batch, token-half).

Self-contained: hardcodes shapes; host folds LN gammas / scales into weights,
computes the tiny style-modulation demod weights, and shards inputs.

Layout: residual stream is dim-major [d(4x128 partitions), tok]. Dim-major
activations serve directly as matmul lhsT (producing token-major outputs) or
rhs (producing dim-major outputs with a transposed-weight lhsT), so no
activation transposes are needed anywhere in the main path.

Precision: prefix fp32 (router argmax must match reference: min top-2 logit
gap ~5e-4), experts bf16 (contributes <1e-3 to final output).
"""
import numpy as np
import ml_dtypes

import concourse.bass as bass
import concourse.mybir as mybir
import concourse.tile as tile
from concourse.bass import ts
from concourse.bass_utils import run_bass_kernel_spmd

F32 = mybir.dt.float32
BF16 = mybir.dt.bfloat16
AX = mybir.AxisListType
OP = mybir.AluOpType
AF = mybir.ActivationFunctionType

DIM = 512
HEADS = 8
HD = 64
B = 4
HW = 1024          # 32*32 tokens per batch
TOK = 512          # own tokens per core
TXT = 77
NE = 8
HID = 2048

TRACE = False
LAST_RESULT = None
_CACHED_NC = [None]

_waitfix_ctr = [0]


def _fix_multiwait(nc, max_waits=1):
    """This walrus build rejects >1 sync wait per instruction; split extras
    onto preceding single-wait NoOps on the same engine."""
    for f in nc.m.functions:
        for blk in f.blocks:
            insts = list(blk.instructions)
            out = []
            changed = False
            for inst in insts:
                si = inst.sync_info
                waits = list(si.on_wait) if (si is not None and si.on_wait) else []
                if len(waits) > max_waits:
                    for w in waits[:-max_waits]:
                        _waitfix_ctr[0] += 1
                        nop = mybir.InstNoOp(
                            name=f"I-waitfix-{_waitfix_ctr[0]}",
                            sync_info=mybir.SyncInfo(on_wait=[w], on_update=[]))
                        nop.engine = inst.engine
                        out.append(nop)
                    si.on_wait = waits[-max_waits:]
                    changed = True
                out.append(inst)
            if changed:
                blk.instructions = out


def _bcast_row(nc, dram_pool, pool, row_ap, nrows, ncols, tag):
    """Replicate a [1, ncols] SBUF row across nrows partitions via DRAM bounce
    (DVE cannot read zero-step partition APs; DMA can, but only from DRAM)."""
    stage = dram_pool.tile([1, ncols], row_ap.dtype, tag=f"{tag}_st")
    nc.sync.dma_start(stage[:], row_ap)
    rep = pool.tile([nrows, ncols], row_ap.dtype, tag=f"{tag}_rep")
    src = bass.AP(tensor=stage[:].tensor, offset=stage[:].offset,
                  ap=[[0, nrows]] + list(stage[:].ap[1:]))
    nc.sync.dma_start(rep[:], src)
    return rep


def _rep_row_pe(nc, pool, psp, onesrow, row_ap, nrows, ncols, tag, psbufs=2):
    """Replicate a [1, ncols] SBUF row (base 0) across nrows partitions using a
    K=1 PE matmul per 512-chunk, landing in SBUF via a DVE copy."""
    rep = pool.tile([nrows, ncols], row_ap.dtype, tag=f"{tag}_rep", name=f"{tag}_rep", bufs=2)
    for n in range(ncols // 512):
        psr = psp.tile([nrows, 512], row_ap.dtype, tag="rep", name="rep", bufs=psbufs)
        nc.tensor.matmul(psr[:], onesrow[0:1, 0:nrows], row_ap[:, ts(n, 512)],
                         start=True, stop=True)
        nc.vector.tensor_copy(rep[:, ts(n, 512)], psr[:])
    return rep


def _layernorm_dim_major(nc, pool, dram_pool, psp, xt, out, ones, onesrow, eps_ap, ntok, tag):
    """LN over the d axis (partitions x 4 tiles) of dim-major xt [128,4,ntok].
    Writes normalized (x-mu)*rstd into out (gamma/beta are folded into the
    consumer weights on the host)."""
    nsp = ntok // 512
    ps_mu = [psp.tile([1, 512], F32, tag=f"{tag}_mu{n}", name=f"{tag}_mu{n}", bufs=1) for n in range(nsp)]
    ps_sq = [psp.tile([1, 512], F32, tag=f"{tag}_sq{n}", name=f"{tag}_sq{n}", bufs=1) for n in range(nsp)]
    for k in range(4):
        for n in range(nsp):
            sq = pool.tile([128, 512], F32, tag=f"{tag}_sqt", name=f"{tag}_sqt", bufs=2)
            nc.vector.tensor_tensor(sq[:], xt[:, k, ts(n, 512)], xt[:, k, ts(n, 512)], OP.mult)
            nc.tensor.matmul(ps_mu[n][:], ones[:], xt[:, k, ts(n, 512)],
                             start=(k == 0), stop=(k == 3), skip_group_check=True)
            nc.tensor.matmul(ps_sq[n][:], ones[:], sq[:],
                             start=(k == 0), stop=(k == 3), skip_group_check=True)
    mu = pool.tile([1, ntok], F32, tag=f"{tag}_murow")
    var = pool.tile([1, ntok], F32, tag=f"{tag}_varrow")
    for n in range(nsp):
        nc.vector.tensor_scalar_mul(mu[:, ts(n, 512)], ps_mu[n][:], 1.0 / DIM)
        nc.vector.tensor_scalar_mul(var[:, ts(n, 512)], ps_sq[n][:], 1.0 / DIM)
    msq = pool.tile([1, ntok], F32, tag=f"{tag}_msqrow")
    nc.vector.tensor_tensor(msq[:], mu[:], mu[:], OP.mult)
    nc.vector.tensor_tensor(var[:], var[:], msq[:], OP.subtract)
    sd = pool.tile([1, ntok], F32, tag=f"{tag}_sdrow")
    nc.scalar.activation(sd[:], var[:], AF.Sqrt, bias=eps_ap, scale=1.0)
    rst = pool.tile([1, ntok], F32, tag=f"{tag}_rstrow")
    nc.vector.reciprocal(rst[:], sd[:])
    mur = _rep_row_pe(nc, pool, psp, onesrow, mu[:], 128, ntok, f"{tag}_mu")
    rstr = _rep_row_pe(nc, pool, psp, onesrow, rst[:], 128, ntok, f"{tag}_rst")
    for k in range(4):
        nc.vector.tensor_tensor(out[:, k, :], xt[:, k, :], mur[:], OP.subtract)
        nc.vector.tensor_tensor(out[:, k, :], out[:, k, :], rstr[:], OP.mult)


def build_nc():
    nc = bass.Bass()
    d = {}
    def inp(name, shape, dt=F32):
        d[name] = nc.dram_tensor(name, shape, dt, kind="ExternalInput")
        return d[name]

    inp("xb", [DIM, HW])
    inp("wtinT", [DIM, DIM])
    inp("wtoutT", [DIM, DIM])
    inp("textT", [DIM, TXT])
    inp("wqkvT", [DIM, 3 * DIM])
    inp("bqkv", [3 * DIM])
    inp("bvrep", [128, DIM])
    inp("cawqT", [DIM, DIM])
    inp("bcaq", [DIM])
    inp("cawkT", [DIM, DIM])
    inp("bcak", [DIM])
    inp("cawvT", [DIM, DIM])
    inp("bcavrep", [128, DIM])
    inp("owsa", [64, HEADS, DIM])
    inp("bosa", [DIM])
    inp("owca", [64, HEADS, DIM])
    inp("boca", [DIM])
    inp("mr", [DIM, NE])
    inp("brrep", [128, NE])
    inp("w1p", [NE, DIM, HID], BF16)
    inp("b1p", [128, NE, HID // 128])
    inp("w2p", [NE, HID, DIM], BF16)
    inp("b2s", [NE, DIM])
    inp("ident", [128, 128])

    xout = nc.dram_tensor("xout", [DIM, TOK], F32, kind="ExternalOutput")
    onehot = nc.dram_tensor("onehot", [TOK, NE], F32, kind="ExternalOutput")

    def loadw(pool, dram_ap, shape, tag, dt=F32, bufs=2):
        t = pool.tile(shape, dt, tag=tag, name=f"{tag}_ld", bufs=bufs)
        nc.sync.dma_start(t[:], dram_ap)
        return t

    with tile.TileContext(nc) as tc:
        with tc.tile_pool(name="outer", bufs=1) as op, \
             tc.tile_pool(name="dram", bufs=2, space="DRAM") as dp:

            # ---- long-lived constants (small) -----------------------------
            TEXTT = op.tile([128, 4, TXT], F32)
            nc.sync.dma_start(TEXTT[:], d["textT"][:].rearrange("(k p) t -> p k t", p=128))
            BQKV = op.tile([128, 12], F32)
            nc.sync.dma_start(BQKV[:], d["bqkv"][:].rearrange("(t p) -> p t", p=128))
            BVREP = op.tile([128, DIM], F32)
            nc.sync.dma_start(BVREP[:], d["bvrep"][:])
            BCAQ = op.tile([128, 4], F32)
            nc.sync.dma_start(BCAQ[:], d["bcaq"][:].rearrange("(t p) -> p t", p=128))
            BCAK = op.tile([128, 4], F32)
            nc.sync.dma_start(BCAK[:], d["bcak"][:].rearrange("(t p) -> p t", p=128))
            BCAVREP = op.tile([128, DIM], F32)
            nc.sync.dma_start(BCAVREP[:], d["bcavrep"][:])
            BOSA = op.tile([128, 4], F32)
            nc.sync.dma_start(BOSA[:], d["bosa"][:].rearrange("(t p) -> p t", p=128))
            BOCA = op.tile([128, 4], F32)
            nc.sync.dma_start(BOCA[:], d["boca"][:].rearrange("(t p) -> p t", p=128))
            MR = op.tile([128, 4, NE], F32)
            nc.sync.dma_start(MR[:], d["mr"][:].rearrange("(k p) e -> p k e", p=128))
            BRREP = op.tile([128, NE], F32)
            nc.sync.dma_start(BRREP[:], d["brrep"][:])
            B1P = op.tile([128, NE, HID // 128], F32)
            nc.sync.dma_start(B1P[:], d["b1p"][:])
            B2S = op.tile([NE, DIM], F32)
            nc.sync.dma_start(B2S[:], d["b2s"][:])
            IDENT = op.tile([128, 128], F32)
            nc.sync.dma_start(IDENT[:], d["ident"][:])
            ONES = op.tile([128, 1], F32)
            nc.vector.memset(ONES[:], 1.0)
            ONESROW = op.tile([1, 128], F32)
            nc.vector.memset(ONESROW[:], 1.0)
            EPS = op.tile([1, 1], F32)
            nc.vector.memset(EPS[:], 1e-5)

            # long-lived activations
            XIN = op.tile([128, 4, HW], F32)      # modconv out, all tokens
            XR = op.tile([128, 4, TOK], F32)      # post-SA residual
            X3 = op.tile([128, 4, TOK], F32)      # post-CA residual
            X3B = op.tile([128, 4, TOK], BF16)    # LN3 out, bf16
            OHT = op.tile([NE, TOK], F32)         # one-hot transposed
            OHTB = op.tile([NE, TOK], BF16)

            with tc.tile_pool(name="poolB", bufs=1) as pb:
                QT = pb.tile([128, 4, TOK], F32)
                KT = pb.tile([128, 4, HW], F32)
                VONE = pb.tile([128, 8, HEADS * 65], F32)

                with tc.tile_pool(name="poolA", bufs=1) as pa, \
                     tc.tile_pool(name="psA", bufs=2, space="PSUM") as psa:
                    X = pa.tile([128, 4, HW], F32)
                    xsrc = d["xb"][:].rearrange("(k p) t -> p k t", p=128)
                    nc.sync.dma_start(X[:, :, 0:512], xsrc[:, :, 0:512])
                    nc.sync.dma_start(X[:, :, 512:1024], xsrc[:, :, 512:1024])
                    # modconv in
                    WTIN = loadw(pa, d["wtinT"][:].rearrange("(k p) o -> p k o", p=128),
                                 [128, 4, DIM], "wmat")
                    for m in range(4):
                        for n in range(2):
                            ps = psa.tile([128, 512], F32, tag="mm", name="mm", bufs=2)
                            for k in range(4):
                                nc.tensor.matmul(ps[:], WTIN[:, k, ts(m, 128)],
                                                 X[:, k, ts(n, 512)],
                                                 start=(k == 0), stop=(k == 3))
                            nc.vector.tensor_copy(XIN[:, m, ts(n, 512)], ps[:])
                    # LN1
                    XN1 = pa.tile([128, 4, HW], F32)
                    _layernorm_dim_major(nc, pa, dp, psa, XIN, XN1, ONES, ONESROW, EPS[:], HW, "ln")
                    # Q
                    WQ = loadw(pa, d["wqkvT"][:, 0:DIM].rearrange("(k p) c -> p k c", p=128),
                               [128, 4, DIM], "wmat")
                    for m in range(4):
                        ps = psa.tile([128, 512], F32, tag="mm", name="mm", bufs=2)
                        for k in range(4):
                            nc.tensor.matmul(ps[:], WQ[:, k, ts(m, 128)], XN1[:, k, 0:TOK],
                                             start=(k == 0), stop=(k == 3))
                        nc.vector.tensor_scalar(QT[:, m, :], ps[:], BQKV[:, m:m + 1],
                                                None, OP.add)
                    # K
                    WK = loadw(pa, d["wqkvT"][:, DIM:2 * DIM].rearrange("(k p) c -> p k c", p=128),
                               [128, 4, DIM], "wmat")
                    for m in range(4):
                        for n in range(2):
                            ps = psa.tile([128, 512], F32, tag="mm", name="mm", bufs=2)
                            for k in range(4):
                                nc.tensor.matmul(ps[:], WK[:, k, ts(m, 128)],
                                                 XN1[:, k, ts(n, 512)],
                                                 start=(k == 0), stop=(k == 3))
                            nc.vector.tensor_scalar(KT[:, m, ts(n, 512)], ps[:],
                                                    BQKV[:, 4 + m:5 + m], None, OP.add)
                    # V token-major with ones column per head
                    WV = loadw(pa, d["wqkvT"][:, 2 * DIM:3 * DIM].rearrange("(k p) c -> p k c", p=128),
                               [128, 4, DIM], "wmat")
                    vview = VONE[:].rearrange("p kt (h c) -> p kt h c", c=65)
                    nc.vector.memset(vview[:, :, :, 64:65], 1.0)
                    for kt in range(8):
                        ps = psa.tile([128, 512], F32, tag="mm", name="mm", bufs=2)
                        for k in range(4):
                            nc.tensor.matmul(ps[:], XN1[:, k, ts(kt, 128)], WV[:, k, :],
                                             start=(k == 0), stop=(k == 3))
                        nc.vector.tensor_tensor(
                            vview[:, kt, :, 0:64],
                            ps[:].rearrange("p (h c) -> p h c", c=64),
                            BVREP[:].rearrange("p (h c) -> p h c", c=64),
                            OP.add)

                # ---- self-attention --------------------------------------
                pb2cm = tc.tile_pool(name="poolB2", bufs=1)
                pb2 = pb2cm.__enter__()
                psbcm = tc.tile_pool(name="psB", bufs=2, space="PSUM")
                psb = psbcm.__enter__()
                OWSA = loadw(pb2, d["owsa"][:], [64, HEADS, DIM], "ow", bufs=1)
                AVHS = pb2.tile([64, HEADS, TOK], F32, tag="avhs", name="avhs")
                AVRAW = pb2.tile([64, HEADS, TOK], F32, tag="avraw", name="avraw")
                RCPS = pb2.tile([1, HEADS, TOK], F32, tag="rcps", name="rcps")
                for hp in range(HEADS // 2):
                    pt = hp
                    exps_pair = []
                    for hh in range(2):
                        h = 2 * hp + hh
                        off = hh * 64
                        EXPS = pb2.tile([128, 8, TOK], F32, tag=f"exps{hh}",
                                        name=f"exps{hh}", bufs=1)
                        exps_pair.append(EXPS)
                    for kt in range(8):
                        for hh in range(2):
                            off = hh * 64
                            ps_s = psb.tile([128, 512], F32, tag="mm", name="mm", bufs=2)
                            nc.tensor.matmul(ps_s[:], KT[off:off + 64, pt, ts(kt, 128)],
                                             QT[off:off + 64, pt, :], start=True, stop=True)
                            nc.scalar.activation(exps_pair[hh][:, kt, :], ps_s[:], AF.Exp)
                    for hh in range(2):
                        h = 2 * hp + hh
                        EXPS = exps_pair[hh]
                        ps_av = psb.tile([65, 512], F32, tag="av", name="av", bufs=2)
                        for kt in range(8):
                            nc.tensor.matmul(ps_av[:], VONE[:, kt, h * 65:(h + 1) * 65],
                                             EXPS[:, kt, :], start=(kt == 0), stop=(kt == 7))
                        nc.vector.reciprocal(RCPS[:, h, :], ps_av[64:65, :])
                        nc.scalar.copy(AVRAW[:, h, :], ps_av[0:64, :])
                for h in range(HEADS):
                    rcpr = _rep_row_pe(nc, pb2, psb, ONESROW, RCPS[:, h, :], 64, TOK, "sarcp")
                    nc.vector.tensor_tensor(AVHS[:, h, :], AVRAW[:, h, :], rcpr[:], OP.mult)
                for m in range(4):
                    ps_o = psb.tile([128, 512], F32, tag="mm", name="mm", bufs=2)
                    for h in range(HEADS):
                        nc.tensor.matmul(ps_o[:], OWSA[:, h, ts(m, 128)], AVHS[:, h, :],
                                         start=(h == 0), stop=(h == 7))
                    nc.vector.scalar_tensor_tensor(XR[:, m, :], ps_o[:], BOSA[:, m:m + 1],
                                                   XIN[:, m, 0:TOK], OP.add, OP.add)
                psbcm.__exit__(None, None, None)
                pb2cm.__exit__(None, None, None)

            # ---- cross-attention + LN3 + router --------------------------
            with tc.tile_pool(name="poolC", bufs=1) as pc, \
                 tc.tile_pool(name="psC", bufs=2, space="PSUM") as psc:
                XN2 = pc.tile([128, 4, TOK], F32)
                _layernorm_dim_major(nc, pc, dp, psc, XR, XN2, ONES, ONESROW, EPS[:], TOK, "ln")
                CAWQ = loadw(pc, d["cawqT"][:].rearrange("(k p) c -> p k c", p=128),
                             [128, 4, DIM], "wmat")
                QTC = pc.tile([128, 4, TOK], F32)
                for m in range(4):
                    ps = psc.tile([128, 512], F32, tag="mm", name="mm", bufs=2)
                    for k in range(4):
                        nc.tensor.matmul(ps[:], CAWQ[:, k, ts(m, 128)], XN2[:, k, :],
                                         start=(k == 0), stop=(k == 3))
                    nc.vector.tensor_scalar(QTC[:, m, :], ps[:], BCAQ[:, m:m + 1],
                                            None, OP.add)
                CAWK = loadw(pc, d["cawkT"][:].rearrange("(k p) c -> p k c", p=128),
                             [128, 4, DIM], "wmat")
                KTC = pc.tile([128, 4, TXT], F32)
                for m in range(4):
                    ps = psc.tile([128, 512], F32, tag="mm", name="mm", bufs=2)
                    for k in range(4):
                        nc.tensor.matmul(ps[:, 0:TXT], CAWK[:, k, ts(m, 128)],
                                         TEXTT[:, k, :], start=(k == 0), stop=(k == 3))
                    nc.vector.tensor_scalar(KTC[:, m, :], ps[:, 0:TXT],
                                            BCAK[:, m:m + 1], None, OP.add)
                CAWV = loadw(pc, d["cawvT"][:].rearrange("(k p) c -> p k c", p=128),
                             [128, 4, DIM], "wmat")
                VONEC = pc.tile([128, HEADS * 65], F32)
                nc.vector.memset(VONEC[:], 0.0)
                vcview = VONEC[:].rearrange("p (h c) -> p h c", c=65)
                nc.vector.memset(vcview[0:TXT, :, 64:65], 1.0)
                ps_v = psc.tile([128, 512], F32, tag="mm", name="mm", bufs=2)
                for k in range(4):
                    nc.tensor.matmul(ps_v[0:TXT, :], TEXTT[:, k, :], CAWV[:, k, :],
                                     start=(k == 0), stop=(k == 3))
                nc.vector.tensor_tensor(
                    vcview[0:TXT, :, 0:64],
                    ps_v[0:TXT, :].rearrange("p (h c) -> p h c", c=64),
                    BCAVREP[0:TXT, :].rearrange("p (h c) -> p h c", c=64),
                    OP.add)
                OWCA = loadw(pc, d["owca"][:], [64, HEADS, DIM], "ow", bufs=1)
                AVHC = pc.tile([64, HEADS, TOK], F32)
                AVRAWC = pc.tile([64, HEADS, TOK], F32, tag="avrawc", name="avrawc")
                RCPSC = pc.tile([1, HEADS, TOK], F32, tag="rcpsc", name="rcpsc")
                for h in range(HEADS):
                    off = (h % 2) * 64
                    pt = h // 2
                    ps_s = psc.tile([128, 512], F32, tag="mm", name="mm", bufs=2)
                    nc.tensor.matmul(ps_s[0:TXT, :], KTC[off:off + 64, pt, :],
                                     QTC[off:off + 64, pt, :], start=True, stop=True)
                    expsc = pc.tile([TXT, TOK], F32, tag="expsc", name="expsc", bufs=2)
                    nc.scalar.activation(expsc[:], ps_s[0:TXT, :], AF.Exp)
                    ps_av = psc.tile([65, 512], F32, tag="av", name="av", bufs=2)
                    nc.tensor.matmul(ps_av[:], vcview[0:TXT, h, :], expsc[:],
                                     start=True, stop=True)
                    nc.vector.reciprocal(RCPSC[:, h, :], ps_av[64:65, :])
                    nc.scalar.copy(AVRAWC[:, h, :], ps_av[0:64, :])
                for h in range(HEADS):
                    rcpr = _rep_row_pe(nc, pc, psc, ONESROW, RCPSC[:, h, :], 64, TOK, "carcp")
                    nc.vector.tensor_tensor(AVHC[:, h, :], AVRAWC[:, h, :], rcpr[:], OP.mult)
                for m in range(4):
                    ps_o = psc.tile([128, 512], F32, tag="mm", name="mm", bufs=2)
                    for h in range(HEADS):
                        nc.tensor.matmul(ps_o[:], OWCA[:, h, ts(m, 128)], AVHC[:, h, :],
                                         start=(h == 0), stop=(h == 7))
                    nc.vector.scalar_tensor_tensor(X3[:, m, :], ps_o[:], BOCA[:, m:m + 1],
                                                   XR[:, m, :], OP.add, OP.add)
                # LN3 + bf16 cast
                XH3 = pc.tile([128, 4, TOK], F32)
                _layernorm_dim_major(nc, pc, dp, psc, X3, XH3, ONES, ONESROW, EPS[:], TOK, "ln")
                for k in range(4):
                    nc.vector.tensor_copy(X3B[:, k, :], XH3[:, k, :])
                # router -> one-hot (token-major out + transposed for masking)
                OH = pc.tile([128, 4, NE], F32)
                for tt in range(4):
                    ps_l = psc.tile([128, NE], F32, tag="mm", name="mm", bufs=2)
                    for k in range(4):
                        nc.tensor.matmul(ps_l[:], XH3[:, k, ts(tt, 128)], MR[:, k, :],
                                         start=(k == 0), stop=(k == 3))
                    logt = pc.tile([128, NE], F32, tag="logt", name="logt")
                    nc.vector.tensor_tensor(logt[:], ps_l[:], BRREP[:], OP.add)
                    rmx = pc.tile([128, 1], F32, tag="rmx", name="rmx")
                    nc.vector.tensor_reduce(rmx[:], logt[:], axis=AX.X, op=OP.max)
                    nc.vector.tensor_scalar(OH[:, tt, :], logt[:], rmx[:], None, OP.is_equal)
                    ps_t = psc.tile([NE, 128], F32, tag="mm", name="mm", bufs=2)
                    nc.tensor.transpose(ps_t[:], OH[:, tt, :], IDENT[:])
                    nc.vector.tensor_copy(OHT[:, ts(tt, 128)], ps_t[:])
                nc.sync.dma_start(onehot[:].rearrange("(t p) e -> p t e", p=128), OH[:])
                nc.vector.tensor_copy(OHTB[:], OHT[:])

            # ---- MoE (dense, masked-hdn, shared PSUM accumulation) -------
            with tc.tile_pool(name="poolD", bufs=1) as pd, \
                 tc.tile_pool(name="psmoe", bufs=1, space="PSUM") as psmoe, \
                 tc.tile_pool(name="psD", bufs=2, space="PSUM") as psd:
                mstage = dp.tile([NE, TOK], BF16, tag="mstage", name="mstage")
                nc.sync.dma_start(mstage[:], OHTB[:])
                MASKS = pd.tile([128, NE, TOK], BF16)
                for e in range(NE):
                    row = mstage[e:e + 1, :]
                    src = bass.AP(tensor=row.tensor, offset=row.offset,
                                  ap=[[0, 128], [1, TOK]])
                    nc.sync.dma_start(MASKS[:, e, :], src)

                MOEPS = [psmoe.tile([128, 512], F32, tag=f"moe{m}", name=f"moe{m}")
                         for m in range(4)]
                for m in range(4):
                    nc.tensor.matmul(MOEPS[m][:], B2S[:, ts(m, 128)], OHT[:],
                                     start=True, stop=False, skip_group_check=True)
                for e in range(NE):
                    HDNB = pd.tile([128, 16, TOK], BF16, tag="hdnb", name="hdnb", bufs=2)
                    for half in range(2):
                        W1H = loadw(pd, d["w1p"][e].rearrange("(k p) h -> p k h", p=128)
                                    [:, :, ts(half, HID // 2)],
                                    [128, 4, HID // 2], "w1h", dt=BF16)
                        for mh in range(8):
                            m = half * 8 + mh
                            ps_h = psd.tile([128, 512], F32, tag="mm", name="mm", bufs=3)
                            for k in range(4):
                                nc.tensor.matmul(ps_h[:], W1H[:, k, ts(mh, 128)],
                                                 X3B[:, k, :],
                                                 start=(k == 0), stop=(k == 3))
                            nc.scalar.activation(HDNB[:, m, :], ps_h[:], AF.Gelu,
                                                 bias=B1P[:, e, m:m + 1], scale=1.0)
                            nc.vector.tensor_tensor(HDNB[:, m, :], HDNB[:, m, :],
                                                    MASKS[:, e, :], OP.mult)
                    for half in range(2):
                        W2H = loadw(pd, d["w2p"][e].rearrange("(k p) o -> p k o", p=128)
                                    [:, ts(half, 8), :],
                                    [128, 8, DIM], "w2h", dt=BF16)
                        for m in range(4):
                            for khh in range(8):
                                kh = half * 8 + khh
                                nc.tensor.matmul(MOEPS[m][:], W2H[:, khh, ts(m, 128)],
                                                 HDNB[:, kh, :], start=False,
                                                 stop=(e == NE - 1 and kh == 15),
                                                 skip_group_check=True)

                # residual + modconv out
                X4 = pd.tile([128, 4, TOK], F32)
                for m in range(4):
                    nc.vector.tensor_tensor(X4[:, m, :], MOEPS[m][:], X3[:, m, :], OP.add)
                WTOUT = loadw(pd, d["wtoutT"][:].rearrange("(k p) o -> p k o", p=128),
                              [128, 4, DIM], "wmat", bufs=1)
                OUT = pd.tile([128, 4, TOK], F32)
                for m in range(4):
                    ps = psd.tile([128, 512], F32, tag="mm", name="mm", bufs=3)
                    for k in range(4):
                        nc.tensor.matmul(ps[:], WTOUT[:, k, ts(m, 128)], X4[:, k, :],
                                         start=(k == 0), stop=(k == 3))
                    nc.vector.tensor_copy(OUT[:, m, :], ps[:])
                nc.sync.dma_start(xout[:].rearrange("(m p) t -> p m t", p=128), OUT[:])

    _fix_multiwait(nc)
    return nc


def _prep_host(i):
    f32 = np.float32
    x = np.asarray(i["x"], f32)
    w = np.asarray(i["w"], f32)
    text = np.asarray(i["text_seq"], f32)
    s = HD ** -0.5

    def modw(weight, mod_w, mod_b):
        style = w @ np.asarray(mod_w, f32).T + np.asarray(mod_b, f32)
        wt = np.asarray(weight, f32)[None] * style[:, None, :]
        wt = wt / np.sqrt((wt ** 2).sum(2, keepdims=True) + 1e-8)
        return np.ascontiguousarray(wt.transpose(0, 2, 1))  # [B, i, o]

    wtinT = modw(i["pin_weight"], i["pin_mod_w"], i["pin_mod_b"])
    wtoutT = modw(i["pout_weight"], i["pout_mod_w"], i["pout_mod_b"])

    g1, b1 = np.asarray(i["ln1_g"], f32), np.asarray(i["ln1_b"], f32)
    Wq, Wk, Wv = np.split(np.asarray(i["sa_in_w"], f32), 3, 0)
    bq, bk, bv = np.split(np.asarray(i["sa_in_b"], f32), 3)
    Wq2 = Wq * g1[None, :] * s
    bq2 = (Wq @ b1 + bq) * s
    Wk2 = Wk * g1[None, :]
    bk2 = Wk @ b1 + bk
    Wv2 = Wv * g1[None, :]
    bv2 = Wv @ b1 + bv
    wqkvT = np.ascontiguousarray(np.concatenate([Wq2, Wk2, Wv2], 0).T)
    bqkv = np.concatenate([bq2, bk2, bv2])
    bvrep = np.tile(bv2[None, :], (128, 1))

    g2, b2l = np.asarray(i["ln2_g"], f32), np.asarray(i["ln2_b"], f32)
    Cq, Ck, Cv = np.split(np.asarray(i["ca_in_w"], f32), 3, 0)
    cbq, cbk, cbv = np.split(np.asarray(i["ca_in_b"], f32), 3)
    cawqT = np.ascontiguousarray((Cq * g2[None, :] * s).T)
    bcaq = (Cq @ b2l + cbq) * s
    cawkT = np.ascontiguousarray(Ck.T)
    bcak = cbk.copy()
    cawvT = np.ascontiguousarray(Cv.T)
    bcavrep = np.tile(cbv[None, :], (128, 1))

    owsa = np.ascontiguousarray(
        np.asarray(i["sa_out_w"], f32).T.reshape(HEADS, 64, DIM).transpose(1, 0, 2))
    bosa = np.asarray(i["sa_out_b"], f32)
    owca = np.ascontiguousarray(
        np.asarray(i["ca_out_w"], f32).T.reshape(HEADS, 64, DIM).transpose(1, 0, 2))
    boca = np.asarray(i["ca_out_b"], f32)

    g3, b3 = np.asarray(i["ln3_g"], f32), np.asarray(i["ln3_b"], f32)
    rf = np.asarray(i["r_feat_mu"], f32)
    rt = np.asarray(i["r_text_mu"], f32)
    rc = np.asarray(i["r_comb_mu"], f32)
    mr = (g3[:, None] * rf) @ rc[:128]
    br_b = (b3 @ rf) @ rc[:128] + (w @ rt) @ rc[128:]     # [B, 8]

    e_w1 = np.asarray(i["e_w1"], f32)
    e_b1 = np.asarray(i["e_b1"], f32)
    e_w2 = np.asarray(i["e_w2"], f32)
    e_b2 = np.asarray(i["e_b2"], f32)
    w1p = np.ascontiguousarray((e_w1 * g3[None, :, None]).astype(ml_dtypes.bfloat16))
    b1fold = e_b1 + np.einsum("d,edh->eh", b3, e_w1)      # [8, 2048]
    b1p = np.ascontiguousarray(b1fold.reshape(NE, HID // 128, 128).transpose(2, 0, 1))
    w2p = np.ascontiguousarray(e_w2.astype(ml_dtypes.bfloat16))
    b2s = np.ascontiguousarray(e_b2)

    ident = np.eye(128, dtype=f32)
    shared = dict(wqkvT=wqkvT, bqkv=bqkv, bvrep=bvrep, cawqT=cawqT, bcaq=bcaq,
                  cawkT=cawkT, bcak=bcak, cawvT=cawvT, bcavrep=bcavrep,
                  owsa=owsa, bosa=bosa, owca=owca, boca=boca, mr=mr,
                  w1p=w1p, b1p=b1p, w2p=w2p, b2s=b2s, ident=ident)
    shared = {k: np.ascontiguousarray(v) for k, v in shared.items()}

    in_maps = []
    for c in range(8):
        b, half = c // 2, c % 2
        hw = x[b].reshape(DIM, HW)
        if half:
            xb = np.concatenate([hw[:, TOK:], hw[:, :TOK]], 1)
        else:
            xb = hw
        m = dict(shared)
        m.update(xb=np.ascontiguousarray(xb),
                 wtinT=wtinT[b], wtoutT=wtoutT[b],
                 textT=np.ascontiguousarray(text[b].T),
                 brrep=np.ascontiguousarray(np.tile(br_b[b][None, :], (128, 1))))
        in_maps.append(m)
    return in_maps


def kernel(**inputs):
    global LAST_RESULT
    in_maps = _prep_host(inputs)
    if _CACHED_NC[0] is None:
        _CACHED_NC[0] = build_nc()
    nc = _CACHED_NC[0]
    try:
        res = run_bass_kernel_spmd(nc, in_maps, core_ids=list(range(8)), trace=TRACE)
    except ModuleNotFoundError:
        res = run_bass_kernel_spmd(nc, in_maps, core_ids=list(range(8)), trace=False)
    LAST_RESULT = res

    x_out = np.zeros((B, DIM, HW), np.float32)
    one_hot = np.zeros((B * HW, NE), np.float32)
    for c in range(8):
        b, half = c // 2, c % 2
        r = res.results[c]
        x_out[b][:, half * TOK:(half + 1) * TOK] = r["xout"]
        one_hot[b * HW + half * TOK: b * HW + (half + 1) * TOK] = r["onehot"]
    return x_out.reshape(B, DIM, 32, 32), one_hot
